# Initial kernel scaffold
#
"""Trainium2 Bass kernel for nn_CubeMoveHead.

Model (from the reference):
    scores = relu([node_features | global_features[batch]] @ W1.T + b1) @ W2.T + b2
    logits[b, rank(node), :] = scores[node]   for cube nodes, rank < MAXC
    out = where(move_mask, logits, -1e9)      -> [B, MAXC*M]

Key structural fact: `batch` is sorted and rank is the running count of cube
nodes within each graph, so the cube nodes of graph b fill slots 0..ncubes_b-1
contiguously.  The scatter is therefore a per-graph compaction, which the host
performs while sharding (8 graphs -> 1 core).  Each core runs a dense 2-layer
MLP over its slot-ordered node features; move_mask + slot-validity are applied
on the host after gathering (the device returns raw fp16 scores).

Device layout per core (8 graphs x PAD slots = S slots, tiles of 448):
  x<c>  [128, 3584]   slot features, feature-major, 5 chunks (sync HWDGE ring)
  w1T   [128, 128]    W1 node-part, transposed (stationary of MM1)
  cT    [128, 8]      per-graph bias = global @ W1_g.T + b1 (folded into relu)
  w2T   [128, 32]     W2.T zero-padded 24->32 (stationary of MM2, col-tiled)
  outP  [128, S/4]    scores, packed so partition 32*t+m = move m of tile t
                      within a 4-tile group, fp16 (scalar HWDGE ring)

Per 448-slot tile i (graph g = i//5):
  psum1 = x_tile @ w1T                       (TensorE, fp16 in / fp32 acc)
  h     = relu(psum1 + c[g])                 (ScalarE/VectorE, pair-drained)
  psum2[32t:32t+24] = h @ w2T                (TensorE, col-tiled 4 tiles/bank)
  out   = fp16(psum2)                        (VectorE copy)

DMA is the bottleneck (one HWDGE ring streams ~215 GB/s at ~55% engine duty),
so inputs and outputs ride separate rings and the mask never leaves the host.
"""

import os
import numpy as np

# ---------------------------------------------------------------- constants
N = 262144
B = 64
D = 128
G = 128
H = 128
M = 24
MAXC = 4096
NEG = np.float32(-1e9)

NCORES = 8
BL = B // NCORES          # graphs per core = 8
PAD = 2240                # slot capacity per graph (max ncubes = 2125 @ seed 0)
S = BL * PAD              # slots per core = 17920
TILE = 448                # matmul moving free dim; PAD/TILE = 5 tiles/graph
TPG = PAD // TILE         # 5
GRP = 4                   # tiles packed per output group (col-tiling of MM2)
NT = S // TILE            # 40 tiles per core
NGRP = NT // GRP          # 10 groups per core
# streamed X chunks, sized in tiles; 4-tile steady chunks keep the DMA
# trigger rate (~0.6us per DMA_DIRECT2D on SyncE) ahead of the data, with
# two small head chunks so the first MM1s start during the ring's cold ramp
CHUNK_TILES = [4] * 10
NCHUNK = len(CHUNK_TILES)
CHUNK_OFF = [sum(CHUNK_TILES[:i]) * TILE for i in range(NCHUNK + 1)]

# matmul dtype: "fp16" (default: 1 cyc/row, ~1e-4 abs err), "bf16", "f32"
MM_DTYPE = os.environ.get("KERNEL_MM_DTYPE", "fp16")

_cache: dict = {}


# ---------------------------------------------------------------- device IR
def _build_bass():
    import concourse.bacc as bacc
    import concourse.mybir as mybir
    from concourse.tile import TileContext

    f32 = mybir.dt.float32
    f16 = mybir.dt.float16
    mdt = {"fp16": mybir.dt.float16, "bf16": mybir.dt.bfloat16,
           "f32": f32}[MM_DTYPE]

    nc = bacc.Bacc("TRN2", target_bir_lowering=False)
    xch = [nc.declare_dram_parameter(f"x{c}", [D, CHUNK_TILES[c] * TILE], mdt,
                                     isOutput=False)
           for c in range(NCHUNK)]
    w1T = nc.declare_dram_parameter("w1T", [D, H], mdt, isOutput=False)
    w2T = nc.declare_dram_parameter("w2T", [H, 32], mdt, isOutput=False)
    cT = nc.declare_dram_parameter("cT", [H, BL], f32, isOutput=False)
    outP = nc.declare_dram_parameter("outP", [128, NGRP * TILE], f16,
                                     isOutput=True)

    relu = mybir.ActivationFunctionType.Relu
    NWARM = 8                 # PE warm-up matmuls overlapped with input DMA

    with TileContext(nc) as tc:
        with (
            tc.tile_pool(name="const", bufs=1) as constp,
            tc.tile_pool(name="xp", bufs=1) as xp,
            tc.tile_pool(name="hp", bufs=8) as hp,
            tc.tile_pool(name="op", bufs=4) as op,
            tc.tile_pool(name="pp", bufs=3, space="PSUM") as ppp,
            tc.tile_pool(name="ps2", bufs=2, space="PSUM") as ps2p,
        ):
            # Everything rides the sync HWDGE ring (Q1): the scalar ring has
            # multi-us startup latency that gated the first drains in v2/v3.
            # Order: x0 first (the stream paces the kernel), then the small
            # weight/bias loads (sandwiched while x0's data streams), then
            # the remaining x chunks; out DMAs queue behind in FIFO order.
            xs = []
            xt = xp.tile([D, CHUNK_TILES[0] * TILE], mdt, tag="x0", name="xt0")
            nc.sync.dma_start(out=xt, in_=xch[0][:, :])
            xs.append(xt)

            w1s = constp.tile([D, H], mdt, tag="w1s")
            nc.sync.dma_start(out=w1s, in_=w1T[:, :])
            w2s = constp.tile([H, 32], mdt, tag="w2s")
            nc.sync.dma_start(out=w2s, in_=w2T[:, :])
            cs = constp.tile([H, BL], f32, tag="cs")
            nc.sync.dma_start(out=cs, in_=cT[:, :])

            for c in range(1, NCHUNK):
                xt = xp.tile([D, CHUNK_TILES[c] * TILE], mdt, tag=f"x{c}",
                             name=f"xt{c}")
                nc.sync.dma_start(out=xt, in_=xch[c][:, :])
                xs.append(xt)

            # warm the PE HAM clock gate while the input DMAs stream in
            wu = constp.tile([128, TILE], mdt, tag="wu")
            nc.gpsimd.memset(wu, 0.0)
            for w in range(NWARM):
                wups = ps2p.tile([128, 512], f32, tag="ps2", name="wups")
                nc.tensor.matmul(wups[:, 0:TILE], wu[:, 0:128], wu,
                                 start=True, stop=True)

            def x_slice(i):
                off = i * TILE
                for c in range(NCHUNK):
                    if off < CHUNK_OFF[c + 1]:
                        r = off - CHUNK_OFF[c]
                        return xs[c][:, r:r + TILE]
                raise AssertionError(i)

            def drain(ht, ps1, p, on_dve):
                """relu(psum_pair + c[g]) -> ht [128, 896]."""
                g0, g1 = (2 * p) // TPG, (2 * p + 1) // TPG
                if on_dve:
                    if g0 == g1:
                        nc.vector.tensor_scalar(
                            ht.rearrange("h (k t) -> h k t", k=2),
                            ps1[:, :, 0:TILE], cs[:, g0:g0 + 1], 0.0,
                            mybir.AluOpType.add, mybir.AluOpType.max)
                    else:
                        for k, g in ((0, g0), (1, g1)):
                            nc.vector.tensor_scalar(
                                ht[:, k * TILE:(k + 1) * TILE],
                                ps1[:, k, 0:TILE], cs[:, g:g + 1], 0.0,
                                mybir.AluOpType.add, mybir.AluOpType.max)
                else:
                    if g0 == g1:
                        nc.scalar.activation(
                            ht.rearrange("h (k t) -> h k t", k=2),
                            ps1[:, :, 0:TILE], relu,
                            bias=cs[:, g0:g0 + 1], scale=1.0)
                    else:
                        for k, g in ((0, g0), (1, g1)):
                            nc.scalar.activation(
                                ht[:, k * TILE:(k + 1) * TILE],
                                ps1[:, k, 0:TILE], relu,
                                bias=cs[:, g:g + 1], scale=1.0)

            # unit = 1 group = 1 chunk = 4 tiles = 2 pairs.  Software-
            # pipelined: unit u's MM2s are emitted AFTER unit u+1's MM1s so
            # slow drains never head-of-line-block the next unit's MM1s on
            # the in-order PE queue.  Fine granularity keeps the PE tracking
            # the DMA stream with ~1-unit latency.
            def emit_mm1(u):
                pair_hts = []
                for pk in range(2):
                    p = 2 * u + pk
                    ps1 = ppp.tile([128, 2, 512], f32, tag="pp", name="ps1")
                    nc.tensor.matmul(ps1[:, 0, 0:TILE], w1s, x_slice(2 * p),
                                     start=True, stop=True)
                    nc.tensor.matmul(ps1[:, 1, 0:TILE], w1s,
                                     x_slice(2 * p + 1),
                                     start=True, stop=True)
                    ht = hp.tile([H, 2 * TILE], mdt, tag="ht", name="ht")
                    # drains split strictly ACT/DVE
                    drain(ht, ps1, p, on_dve=(pk == 1))
                    pair_hts.append(ht)
                return pair_hts

            def emit_mm2(u, pair_hts):
                ps2 = ps2p.tile([128, 512], f32, tag="ps2", name="ps2")
                for t in range(GRP):
                    i = 4 * u + t
                    ht = pair_hts[t // 2]
                    rhs = ht[:, (i % 2) * TILE:(i % 2 + 1) * TILE]
                    nc.tensor.matmul(ps2[32 * t:32 * t + 32, 0:TILE],
                                     w2s, rhs, start=True, stop=True,
                                     tile_position=(0, 32 * t))
                ot = op.tile([128, TILE], f16, tag="ot", name="ot")
                if u % 2 == 0:
                    nc.scalar.copy(ot, ps2[:, 0:TILE])
                else:
                    nc.vector.tensor_scalar_add(ot, ps2[:, 0:TILE], 0.0)
                nc.sync.dma_start(out=outP[:, u * TILE:(u + 1) * TILE],
                                  in_=ot)

            # 2-unit MM2 lag: by the time MM2(u) reaches the head of the PE
            # queue its drains finished long ago, so the queue never blocks
            pend: list = []
            for u in range(NGRP):
                pend.append((u, emit_mm1(u)))
                if len(pend) > 2:
                    uu, hts = pend.pop(0)
                    emit_mm2(uu, hts)
            for uu, hts in pend:
                emit_mm2(uu, hts)

    nc.compile()
    return nc


def _get_nc():
    if "nc" not in _cache:
        _cache["nc"] = _build_bass()
    return _cache["nc"]


# ------------------------------------------------------------ numpy fallback
def _numpy_reference(node_features, global_features, W1, b1, W2, b2,
                     cube_mask, batch, move_mask):
    g_exp = global_features[batch]
    x = np.concatenate([node_features, g_exp], axis=-1)
    h = np.maximum(x @ W1.T + b1, 0.0)
    scores = h @ W2.T + b2
    cm = cube_mask.astype(np.int64)
    excl = np.cumsum(cm) - cm
    seg_base = np.full(B, np.iinfo(np.int64).max)
    np.minimum.at(seg_base, batch, excl)
    rank = excl - seg_base[batch]
    valid = cube_mask & (rank < MAXC)
    logits = np.full((B, MAXC, M), NEG, dtype=np.float32)
    logits[batch[valid], rank[valid]] = scores[valid]
    logits = np.where(move_mask, logits, NEG)
    return logits.reshape(B, -1).astype(np.float32)


# ------------------------------------------------------------------- kernel
def kernel(node_features, global_features, W1, b1, W2, b2,
           cube_mask, batch, move_mask):
    node_features = np.ascontiguousarray(np.asarray(node_features, np.float32))
    global_features = np.asarray(global_features, np.float32)
    W1 = np.asarray(W1, np.float32)
    b1 = np.asarray(b1, np.float32)
    W2 = np.asarray(W2, np.float32)
    b2 = np.asarray(b2, np.float32)
    cube_mask = np.asarray(cube_mask, bool)
    batch = np.asarray(batch, np.int64)
    move_mask = np.asarray(move_mask, bool)

    # --- ranks: cube nodes of each graph fill slots 0..ncubes-1 in order
    cube_idx = np.flatnonzero(cube_mask)
    g = batch[cube_idx]
    counts = np.bincount(g, minlength=B)
    if counts.max() > PAD:
        # a graph overflows the device slot capacity: bail to exact numpy
        return _numpy_reference(node_features, global_features, W1, b1, W2,
                                b2, cube_mask, batch, move_mask)
    starts = np.concatenate(([0], np.cumsum(counts)[:-1]))
    within = np.arange(cube_idx.size, dtype=np.int64) - starts[g]

    if MM_DTYPE == "bf16":
        import ml_dtypes
        mm_np = ml_dtypes.bfloat16
    elif MM_DTYPE == "fp16":
        mm_np = np.float16
    else:
        mm_np = np.float32

    Xslot = np.zeros((B, PAD, D), mm_np)
    Xslot[g, within] = node_features[cube_idx].astype(mm_np)
    slotvalid = np.zeros((B, PAD), bool)
    slotvalid[g, within] = True

    # --- fold the global-feature contribution + b1 into a per-graph bias
    cb = (global_features @ W1[:, D:].T + b1).astype(np.float32)    # [B, H]

    w1T = np.ascontiguousarray(W1[:, :D].T.astype(mm_np))           # [128,128]
    w2T = np.zeros((H, 32), mm_np)
    w2T[:, :M] = W2.T.astype(mm_np)

    nc = _get_nc()
    in_maps = []
    for c in range(NCORES):
        gs = slice(c * BL, (c + 1) * BL)
        xT = np.ascontiguousarray(Xslot[gs].reshape(S, D).T)        # [128, S]
        im = {
            "w1T": w1T,
            "w2T": w2T,
            "cT": np.ascontiguousarray(cb[gs].T),
        }
        for ch in range(NCHUNK):
            im[f"x{ch}"] = np.ascontiguousarray(
                xT[:, CHUNK_OFF[ch]:CHUNK_OFF[ch + 1]])
        in_maps.append(im)

    from concourse.bass_utils import run_bass_kernel_spmd
    res = run_bass_kernel_spmd(nc, in_maps, list(range(NCORES)))
    _cache["last_result"] = res

    # --- unshard: raw fp16 scores -> fp32, mask applied host-side
    mask_eff = move_mask[:, :PAD, :] & slotvalid[:, :, None]        # [B,PAD,M]
    scores = np.empty((B, PAD, M), dtype=np.float32)
    for c in range(NCORES):
        gs = slice(c * BL, (c + 1) * BL)
        outP = np.asarray(res.results[c]["outP"])
        o = outP.reshape(GRP, 32, NGRP, TILE)[:, :M]                # t,m,j,col
        scores[gs] = o.transpose(2, 0, 3, 1).reshape(
            BL, PAD, M).astype(np.float32)

    if np.any(b2):
        scores += b2[None, None, :].astype(np.float32)

    full = np.full((B, MAXC, M), NEG, dtype=np.float32)
    full[:, :PAD] = np.where(mask_eff, scores, NEG)
    return full.reshape(B, MAXC * M)



# revision 26
# speedup vs baseline: 1.0749x; 1.0749x over previous
"""Trainium2 Bass kernel for nn_CubeMoveHead.

Model (from the reference):
    scores = relu([node_features | global_features[batch]] @ W1.T + b1) @ W2.T + b2
    logits[b, rank(node), :] = scores[node]   for cube nodes, rank < MAXC
    out = where(move_mask, logits, -1e9)      -> [B, MAXC*M]

Key structural fact: `batch` is sorted and rank is the running count of cube
nodes within each graph, so the cube nodes of graph b fill slots 0..ncubes_b-1
contiguously.  The scatter is therefore a per-graph compaction, which the host
performs while sharding (8 graphs -> 1 core).  Each core runs a dense 2-layer
MLP over its slot-ordered node features; move_mask + slot-validity are applied
on the host after gathering (the device returns raw fp16 scores).

Device layout per core (8 graphs x PAD slots = S slots, tiles of 448):
  x<c>  [128, *]      slot features, feature-major, 9 chunks, all queued up
                      front on the sync HWDGE ring (SBUF holds the full 4.6MB)
  w1T   [128, 128]    W1 node-part, transposed (stationary of MM1)
  cT    [128, 8]      per-graph bias = global @ W1_g.T + b1 (folded into relu)
  w2T   [128, 32]     W2.T zero-padded 24->32 (stationary of MM2, col-tiled)
  outP  [128, S/4]    scores, packed so partition 32*t+m = move m of tile t
                      within a 4-tile group, fp16, via the gpsimd SWDGE ring

Per 448-slot tile i (graph g = i//5):
  psum1 = x_tile @ w1T                       (TensorE, fp16 in / fp32 acc)
  h     = relu(psum1 + c[g])                 (ScalarE/VectorE, pair-drained)
  psum2[32t:32t+24] = h @ w2T                (TensorE, col-tiled, concurrent)
  out   = fp16(psum2)                        (896-col copies, alt ACT/DVE)

Bottleneck ordering (measured): total DMA 5.74MB at ~300-358GB/s shared
HBM budget (~16us) > PSUM evacuation on ACT+DVE (~13us) > PE (~11us warm).
Inputs and outputs ride separate DMA queues so neither blocks the other.
"""

import os
import numpy as np

# ---------------------------------------------------------------- constants
N = 262144
B = 64
D = 128
G = 128
H = 128
M = 24
MAXC = 4096
NEG = np.float32(-1e9)

NCORES = 8
BL = B // NCORES          # graphs per core = 8
PAD = 2240                # slot capacity per graph (max ncubes = 2125 @ seed 0)
S = BL * PAD              # slots per core = 17920
TILE = 448                # matmul moving free dim; PAD/TILE = 5 tiles/graph
TPG = PAD // TILE         # 5
GRP = 4                   # tiles packed per output group (col-tiling of MM2)
NT = S // TILE            # 40 tiles per core
NGRP = NT // GRP          # 10 groups (units) per core
NPAIR = NGRP // 2         # 5 output pairs (2 units per 896-col out DMA)
# streamed X chunks, sized in tiles; small head chunks so the first MM1s
# start during the DMA ring's cold ramp, big steady chunks (>=64KB/partition
# line amortizes descriptor overhead), small tail chunks so the last MM1s
# start as soon as possible
CHUNK_TILES = [2, 6, 8, 8, 8, 5, 3]
NCHUNK = len(CHUNK_TILES)
CHUNK_OFF = [sum(CHUNK_TILES[:i]) * TILE for i in range(NCHUNK + 1)]
assert CHUNK_OFF[-1] == S

# matmul dtype: "fp16" (default: ~1e-4 abs err), "bf16", "f32"
MM_DTYPE = os.environ.get("KERNEL_MM_DTYPE", "fp16")

_cache: dict = {}


# ---------------------------------------------------------------- device IR
def _build_bass():
    import concourse.bacc as bacc
    import concourse.mybir as mybir
    from concourse.tile import TileContext

    f32 = mybir.dt.float32
    f16 = mybir.dt.float16
    mdt = {"fp16": mybir.dt.float16, "bf16": mybir.dt.bfloat16,
           "f32": f32}[MM_DTYPE]

    nc = bacc.Bacc("TRN2", target_bir_lowering=False)
    xch = [nc.declare_dram_parameter(f"x{c}", [D, CHUNK_TILES[c] * TILE], mdt,
                                     isOutput=False)
           for c in range(NCHUNK)]
    w1T = nc.declare_dram_parameter("w1T", [D, H], mdt, isOutput=False)
    w2T = nc.declare_dram_parameter("w2T", [H, 32], mdt, isOutput=False)
    cT = nc.declare_dram_parameter("cT", [H, BL], f32, isOutput=False)
    outP = nc.declare_dram_parameter("outP", [128, NGRP * TILE], f16,
                                     isOutput=True)

    relu = mybir.ActivationFunctionType.Relu
    # PE warm-ups before the first MM1: during the DMA chase phase the PE is
    # saturated even at the cold 1.2GHz rate, so it flips the HAM clock gate
    # to 8/8 on its own; warm-ups only delay the first MM1.
    NWARM = int(os.environ.get("KERNEL_NWARM", "10"))

    with TileContext(nc) as tc:
        with (
            tc.tile_pool(name="const", bufs=1) as constp,
            tc.tile_pool(name="xp", bufs=1) as xp,
            tc.tile_pool(name="hp", bufs=5) as hp,
            tc.tile_pool(name="op", bufs=10) as op,
            tc.tile_pool(name="pp", bufs=3, space="PSUM") as ppp,
            tc.tile_pool(name="ps2", bufs=2, space="PSUM") as ps2p,
        ):
            # All DMAs ride the sync HWDGE ring; inputs are queued up front
            # (SBUF holds everything), outputs trigger as score pairs are
            # copied.  Order: w1 (needed first), x0, then the small w2/cT
            # loads, then the remaining x chunks.
            # x1 triggers before the small w2/cT loads: the ~0.65us trigger
            # rate gates the stream's cold start and x1 is on the critical
            # path of the first MM1s, while w2/cT are not needed until the
            # first MM2/drain several us later
            w1s = constp.tile([D, H], mdt, tag="w1s")
            nc.sync.dma_start(out=w1s, in_=w1T[:, :])

            xs = []
            for c in range(2):
                xt = xp.tile([D, CHUNK_TILES[c] * TILE], mdt, tag=f"x{c}",
                             name=f"xt{c}")
                nc.sync.dma_start(out=xt, in_=xch[c][:, :])
                xs.append(xt)

            w2s = constp.tile([H, 32], mdt, tag="w2s")
            nc.sync.dma_start(out=w2s, in_=w2T[:, :])
            cs = constp.tile([H, BL], f32, tag="cs")
            nc.sync.dma_start(out=cs, in_=cT[:, :])

            for c in range(2, NCHUNK):
                xt = xp.tile([D, CHUNK_TILES[c] * TILE], mdt, tag=f"x{c}",
                             name=f"xt{c}")
                nc.sync.dma_start(out=xt, in_=xch[c][:, :])
                xs.append(xt)

            # warm the PE HAM clock gate while the first chunks stream in
            wu = constp.tile([128, TILE], mdt, tag="wu")
            nc.gpsimd.memset(wu, 0.0)
            wups = ps2p.tile([128, 512], f32, tag="ps2", name="wups")
            for w in range(NWARM):
                nc.tensor.matmul(wups[:, 0:TILE], wu[:, 0:128], wu,
                                 start=True, stop=True)

            def x_slice(i):
                off = i * TILE
                for c in range(NCHUNK):
                    if off < CHUNK_OFF[c + 1]:
                        r = off - CHUNK_OFF[c]
                        return xs[c][:, r:r + TILE]
                raise AssertionError(i)

            ANY_DRAIN = os.environ.get("KERNEL_ANY_DRAIN", "1") == "1"

            def drain(ht2, ps1, p, on_dve):
                """relu(psum_pair + c[g]) -> ht2 [128, 2, 448] view."""
                g0, g1 = (2 * p) // TPG, (2 * p + 1) // TPG
                if ANY_DRAIN:
                    if g0 == g1:
                        nc.any.tensor_scalar(
                            ht2, ps1[:, :, 0:TILE], cs[:, g0:g0 + 1], 0.0,
                            mybir.AluOpType.add, mybir.AluOpType.max)
                    else:
                        for k, g in ((0, g0), (1, g1)):
                            nc.any.tensor_scalar(
                                ht2[:, k], ps1[:, k, 0:TILE],
                                cs[:, g:g + 1], 0.0,
                                mybir.AluOpType.add, mybir.AluOpType.max)
                elif on_dve:
                    if g0 == g1:
                        nc.vector.tensor_scalar(
                            ht2, ps1[:, :, 0:TILE], cs[:, g0:g0 + 1], 0.0,
                            mybir.AluOpType.add, mybir.AluOpType.max)
                    else:
                        for k, g in ((0, g0), (1, g1)):
                            nc.vector.tensor_scalar(
                                ht2[:, k], ps1[:, k, 0:TILE],
                                cs[:, g:g + 1], 0.0,
                                mybir.AluOpType.add, mybir.AluOpType.max)
                else:
                    if g0 == g1:
                        nc.scalar.activation(
                            ht2, ps1[:, :, 0:TILE], relu,
                            bias=cs[:, g0:g0 + 1], scale=1.0)
                    else:
                        for k, g in ((0, g0), (1, g1)):
                            nc.scalar.activation(
                                ht2[:, k], ps1[:, k, 0:TILE], relu,
                                bias=cs[:, g:g + 1], scale=1.0)

            # unit = 4 tiles = 2 pairs.  Software-pipelined with a 1-unit
            # MM2 lag: unit u's MM2s are emitted AFTER unit u+1's MM1s so
            # drains never head-of-line-block the next unit's MM1s on the
            # in-order PE queue.
            def emit_mm1(u):
                ht = hp.tile([H, 4, TILE], mdt, tag="ht", name="ht")
                for pk in range(2):
                    p = 2 * u + pk
                    ps1 = ppp.tile([128, 2, 512], f32, tag="pp", name="ps1")
                    nc.tensor.matmul(ps1[:, 0, 0:TILE], w1s, x_slice(2 * p),
                                     start=True, stop=True)
                    nc.tensor.matmul(ps1[:, 1, 0:TILE], w1s,
                                     x_slice(2 * p + 1),
                                     start=True, stop=True)
                    # drains split strictly ACT/DVE
                    drain(ht[:, 2 * pk:2 * pk + 2], ps1, p,
                          on_dve=(pk == 1))
                return ht

            ps2_tiles = {}

            def emit_mm2(u, ht):
                ps2 = ps2p.tile([128, 512], f32, tag="ps2", name="ps2")
                ps2_tiles[u] = ps2
                for t in range(GRP):
                    nc.tensor.matmul(ps2[32 * t:32 * t + 32, 0:TILE],
                                     w2s, ht[:, t], start=True, stop=True,
                                     tile_position=(0, 32 * t))

            def emit_copy_dma(u):
                """fp16(ps2 of unit u) -> ot [128, 448] -> outP, sync ring."""
                ps2 = ps2_tiles.pop(u)
                ot = op.tile([128, TILE], f16, tag="ot", name="ot")
                # any-engine: the Tile scheduler gap-fills the idler of
                # ACT/DVE, auto-balancing the evacuation load per unit
                nc.any.tensor_copy(ot, ps2[:, 0:TILE])
                nc.sync.dma_start(out=outP[:, u * TILE:(u + 1) * TILE],
                                  in_=ot)

            LAG = int(os.environ.get("KERNEL_MM2_LAG", "1"))
            pend: list = []
            for u in range(NGRP):
                pend.append((u, emit_mm1(u)))
                if len(pend) > LAG:
                    uu, hts = pend.pop(0)
                    emit_mm2(uu, hts)
                    if uu >= 1:
                        emit_copy_dma(uu - 1)
            # tail: copy(8) runs on ACT while MM2(9) runs on the PE
            for uu, hts in pend:
                emit_copy_dma(uu - 1)
                emit_mm2(uu, hts)
            emit_copy_dma(NGRP - 1)

    nc.compile()
    return nc


def _get_nc():
    if "nc" not in _cache:
        _cache["nc"] = _build_bass()
    return _cache["nc"]


# ------------------------------------------------------------ numpy fallback
def _numpy_reference(node_features, global_features, W1, b1, W2, b2,
                     cube_mask, batch, move_mask):
    g_exp = global_features[batch]
    x = np.concatenate([node_features, g_exp], axis=-1)
    h = np.maximum(x @ W1.T + b1, 0.0)
    scores = h @ W2.T + b2
    cm = cube_mask.astype(np.int64)
    excl = np.cumsum(cm) - cm
    seg_base = np.full(B, np.iinfo(np.int64).max)
    np.minimum.at(seg_base, batch, excl)
    rank = excl - seg_base[batch]
    valid = cube_mask & (rank < MAXC)
    logits = np.full((B, MAXC, M), NEG, dtype=np.float32)
    logits[batch[valid], rank[valid]] = scores[valid]
    logits = np.where(move_mask, logits, NEG)
    return logits.reshape(B, -1).astype(np.float32)


# ------------------------------------------------------------------- kernel
def kernel(node_features, global_features, W1, b1, W2, b2,
           cube_mask, batch, move_mask):
    node_features = np.ascontiguousarray(np.asarray(node_features, np.float32))
    global_features = np.asarray(global_features, np.float32)
    W1 = np.asarray(W1, np.float32)
    b1 = np.asarray(b1, np.float32)
    W2 = np.asarray(W2, np.float32)
    b2 = np.asarray(b2, np.float32)
    cube_mask = np.asarray(cube_mask, bool)
    batch = np.asarray(batch, np.int64)
    move_mask = np.asarray(move_mask, bool)

    # --- ranks: cube nodes of each graph fill slots 0..ncubes-1 in order
    cube_idx = np.flatnonzero(cube_mask)
    g = batch[cube_idx]
    counts = np.bincount(g, minlength=B)
    if counts.max() > PAD:
        # a graph overflows the device slot capacity: bail to exact numpy
        return _numpy_reference(node_features, global_features, W1, b1, W2,
                                b2, cube_mask, batch, move_mask)
    starts = np.concatenate(([0], np.cumsum(counts)[:-1]))
    within = np.arange(cube_idx.size, dtype=np.int64) - starts[g]

    if MM_DTYPE == "bf16":
        import ml_dtypes
        mm_np = ml_dtypes.bfloat16
    elif MM_DTYPE == "fp16":
        mm_np = np.float16
    else:
        mm_np = np.float32

    Xslot = np.zeros((B, PAD, D), mm_np)
    Xslot[g, within] = node_features[cube_idx].astype(mm_np)
    slotvalid = np.zeros((B, PAD), bool)
    slotvalid[g, within] = True

    # --- fold the global-feature contribution + b1 into a per-graph bias
    cb = (global_features @ W1[:, D:].T + b1).astype(np.float32)    # [B, H]

    w1T = np.ascontiguousarray(W1[:, :D].T.astype(mm_np))           # [128,128]
    w2T = np.zeros((H, 32), mm_np)
    w2T[:, :M] = W2.T.astype(mm_np)

    nc = _get_nc()
    in_maps = []
    for c in range(NCORES):
        gs = slice(c * BL, (c + 1) * BL)
        xT = np.ascontiguousarray(Xslot[gs].reshape(S, D).T)        # [128, S]
        im = {
            "w1T": w1T,
            "w2T": w2T,
            "cT": np.ascontiguousarray(cb[gs].T),
        }
        for ch in range(NCHUNK):
            im[f"x{ch}"] = np.ascontiguousarray(
                xT[:, CHUNK_OFF[ch]:CHUNK_OFF[ch + 1]])
        in_maps.append(im)

    from concourse.bass_utils import run_bass_kernel_spmd
    res = run_bass_kernel_spmd(nc, in_maps, list(range(NCORES)))
    _cache["last_result"] = res

    # --- unshard: raw fp16 scores -> fp32, mask applied host-side
    mask_eff = move_mask[:, :PAD, :] & slotvalid[:, :, None]        # [B,PAD,M]
    scores = np.empty((B, PAD, M), dtype=np.float32)
    for c in range(NCORES):
        gs = slice(c * BL, (c + 1) * BL)
        outP = np.asarray(res.results[c]["outP"])
        o = outP.reshape(GRP, 32, NGRP, TILE)[:, :M]                # t,m,j,col
        scores[gs] = o.transpose(2, 0, 3, 1).reshape(
            BL, PAD, M).astype(np.float32)

    if np.any(b2):
        scores += b2[None, None, :].astype(np.float32)

    full = np.full((B, MAXC, M), NEG, dtype=np.float32)
    full[:, :PAD] = np.where(mask_eff, scores, NEG)
    return full.reshape(B, MAXC * M)


# revision 27
# speedup vs baseline: 1.1124x; 1.0348x over previous
"""Trainium2 Bass kernel for nn_CubeMoveHead.

Model (from the reference):
    scores = relu([node_features | global_features[batch]] @ W1.T + b1) @ W2.T + b2
    logits[b, rank(node), :] = scores[node]   for cube nodes, rank < MAXC
    out = where(move_mask, logits, -1e9)      -> [B, MAXC*M]

Key structural fact: `batch` is sorted and rank is the running count of cube
nodes within each graph, so the cube nodes of graph b fill slots 0..ncubes_b-1
contiguously.  The scatter is therefore a per-graph compaction, which the host
performs while sharding (8 graphs -> 1 core).  Each core runs a dense 2-layer
MLP over its slot-ordered node features; move_mask + slot-validity are applied
on the host after gathering (the device returns raw fp16 scores).

Device layout per core (8 graphs x PAD slots = S slots, tiles of 448):
  x<c>  [128, *]      slot features, feature-major, 9 chunks, all queued up
                      front on the sync HWDGE ring (SBUF holds the full 4.6MB)
  w1T   [128, 128]    W1 node-part, transposed (stationary of MM1)
  cT    [128, 8]      per-graph bias = global @ W1_g.T + b1 (folded into relu)
  w2T   [128, 32]     W2.T zero-padded 24->32 (stationary of MM2, col-tiled)
  outP  [128, S/4]    scores, packed so partition 32*t+m = move m of tile t
                      within a 4-tile group, fp16, via the gpsimd SWDGE ring

Per 448-slot tile i (graph g = i//5):
  psum1 = x_tile @ w1T                       (TensorE, fp16 in / fp32 acc)
  h     = relu(psum1 + c[g])                 (ScalarE/VectorE, pair-drained)
  psum2[32t:32t+24] = h @ w2T                (TensorE, col-tiled, concurrent)
  out   = fp16(psum2)                        (896-col copies, alt ACT/DVE)

Bottleneck ordering (measured): total DMA 5.74MB at ~300-358GB/s shared
HBM budget (~16us) > PSUM evacuation on ACT+DVE (~13us) > PE (~11us warm).
Inputs and outputs ride separate DMA queues so neither blocks the other.
"""

import os
import numpy as np

# ---------------------------------------------------------------- constants
N = 262144
B = 64
D = 128
G = 128
H = 128
M = 24
MAXC = 4096
NEG = np.float32(-1e9)

NCORES = 8
BL = B // NCORES          # graphs per core = 8
PAD = 2240                # slot capacity per graph (max ncubes = 2125 @ seed 0)
S = BL * PAD              # slots per core = 17920
TILE = 448                # matmul moving free dim; PAD/TILE = 5 tiles/graph
TPG = PAD // TILE         # 5
GRP = 4                   # tiles packed per output group (col-tiling of MM2)
NT = S // TILE            # 40 tiles per core
NGRP = NT // GRP          # 10 groups (units) per core
NPAIR = NGRP // 2         # 5 output pairs (2 units per 896-col out DMA)
# streamed X chunks, sized in tiles; small head chunks so the first MM1s
# start during the DMA ring's cold ramp, big steady chunks (>=64KB/partition
# line amortizes descriptor overhead), small tail chunks so the last MM1s
# start as soon as possible
CHUNK_TILES = [2, 6, 8, 8, 8, 5, 3]
NCHUNK = len(CHUNK_TILES)
CHUNK_OFF = [sum(CHUNK_TILES[:i]) * TILE for i in range(NCHUNK + 1)]
assert CHUNK_OFF[-1] == S

# matmul dtype: "fp16" (default: ~1e-4 abs err), "bf16", "f32"
MM_DTYPE = os.environ.get("KERNEL_MM_DTYPE", "fp16")

_cache: dict = {}


# ---------------------------------------------------------------- device IR
def _build_bass():
    import concourse.bacc as bacc
    import concourse.mybir as mybir
    from concourse.tile import TileContext

    f32 = mybir.dt.float32
    f16 = mybir.dt.float16
    mdt = {"fp16": mybir.dt.float16, "bf16": mybir.dt.bfloat16,
           "f32": f32}[MM_DTYPE]

    nc = bacc.Bacc("TRN2", target_bir_lowering=False)
    xch = [nc.declare_dram_parameter(f"x{c}", [D, CHUNK_TILES[c] * TILE], mdt,
                                     isOutput=False)
           for c in range(NCHUNK)]
    w1T = nc.declare_dram_parameter("w1T", [D, H], mdt, isOutput=False)
    w2T = nc.declare_dram_parameter("w2T", [H, 32], mdt, isOutput=False)
    cT = nc.declare_dram_parameter("cT", [H, BL], f32, isOutput=False)
    outP = nc.declare_dram_parameter("outP", [128, NGRP * TILE], f16,
                                     isOutput=True)

    relu = mybir.ActivationFunctionType.Relu
    # PE warm-ups before the first MM1: during the DMA chase phase the PE is
    # saturated even at the cold 1.2GHz rate, so it flips the HAM clock gate
    # to 8/8 on its own; warm-ups only delay the first MM1.
    NWARM = int(os.environ.get("KERNEL_NWARM", "10"))

    with TileContext(nc) as tc:
        with (
            tc.tile_pool(name="const", bufs=1) as constp,
            tc.tile_pool(name="xp", bufs=1) as xp,
            tc.tile_pool(name="hp", bufs=5) as hp,
            tc.tile_pool(name="op", bufs=10) as op,
            tc.tile_pool(name="pp", bufs=3, space="PSUM") as ppp,
            tc.tile_pool(name="ps2", bufs=2, space="PSUM") as ps2p,
        ):
            # All DMAs ride the sync HWDGE ring; inputs are queued up front
            # (SBUF holds everything), outputs trigger as score pairs are
            # copied.  Order: w1 (needed first), x0, then the small w2/cT
            # loads, then the remaining x chunks.
            # x1 triggers before the small w2/cT loads: the ~0.65us trigger
            # rate gates the stream's cold start and x1 is on the critical
            # path of the first MM1s, while w2/cT are not needed until the
            # first MM2/drain several us later
            w1s = constp.tile([D, H], mdt, tag="w1s")
            nc.sync.dma_start(out=w1s, in_=w1T[:, :])

            xs = []
            for c in range(2):
                xt = xp.tile([D, CHUNK_TILES[c] * TILE], mdt, tag=f"x{c}",
                             name=f"xt{c}")
                nc.sync.dma_start(out=xt, in_=xch[c][:, :])
                xs.append(xt)

            w2s = constp.tile([H, 32], mdt, tag="w2s")
            nc.sync.dma_start(out=w2s, in_=w2T[:, :])
            cs = constp.tile([H, BL], f32, tag="cs")
            nc.sync.dma_start(out=cs, in_=cT[:, :])

            for c in range(2, NCHUNK):
                xt = xp.tile([D, CHUNK_TILES[c] * TILE], mdt, tag=f"x{c}",
                             name=f"xt{c}")
                nc.sync.dma_start(out=xt, in_=xch[c][:, :])
                xs.append(xt)

            # warm the PE HAM clock gate while the first chunks stream in
            wu = constp.tile([128, TILE], mdt, tag="wu")
            nc.gpsimd.memset(wu, 0.0)
            wups = ps2p.tile([128, 512], f32, tag="ps2", name="wups")
            for w in range(NWARM):
                nc.tensor.matmul(wups[:, 0:TILE], wu[:, 0:128], wu,
                                 start=True, stop=True)

            def x_slice(i):
                off = i * TILE
                for c in range(NCHUNK):
                    if off < CHUNK_OFF[c + 1]:
                        r = off - CHUNK_OFF[c]
                        return xs[c][:, r:r + TILE]
                raise AssertionError(i)

            ANY_DRAIN = os.environ.get("KERNEL_ANY_DRAIN", "1") == "1"

            def drain(ht2, ps1, p, on_dve):
                """relu(psum_pair + c[g]) -> ht2 [128, 2, 448] view."""
                g0, g1 = (2 * p) // TPG, (2 * p + 1) // TPG
                if ANY_DRAIN:
                    if g0 == g1 and p == 2 * NGRP - 1:
                        # last pair: split across both engines so the final
                        # MM2/copy/DMA chain starts ~0.5us earlier
                        for k, g in ((0, g0), (1, g1)):
                            nc.any.tensor_scalar(
                                ht2[:, k], ps1[:, k, 0:TILE],
                                cs[:, g:g + 1], 0.0,
                                mybir.AluOpType.add, mybir.AluOpType.max)
                    elif g0 == g1:
                        nc.any.tensor_scalar(
                            ht2, ps1[:, :, 0:TILE], cs[:, g0:g0 + 1], 0.0,
                            mybir.AluOpType.add, mybir.AluOpType.max)
                    else:
                        for k, g in ((0, g0), (1, g1)):
                            nc.any.tensor_scalar(
                                ht2[:, k], ps1[:, k, 0:TILE],
                                cs[:, g:g + 1], 0.0,
                                mybir.AluOpType.add, mybir.AluOpType.max)
                elif on_dve:
                    if g0 == g1:
                        nc.vector.tensor_scalar(
                            ht2, ps1[:, :, 0:TILE], cs[:, g0:g0 + 1], 0.0,
                            mybir.AluOpType.add, mybir.AluOpType.max)
                    else:
                        for k, g in ((0, g0), (1, g1)):
                            nc.vector.tensor_scalar(
                                ht2[:, k], ps1[:, k, 0:TILE],
                                cs[:, g:g + 1], 0.0,
                                mybir.AluOpType.add, mybir.AluOpType.max)
                else:
                    if g0 == g1:
                        nc.scalar.activation(
                            ht2, ps1[:, :, 0:TILE], relu,
                            bias=cs[:, g0:g0 + 1], scale=1.0)
                    else:
                        for k, g in ((0, g0), (1, g1)):
                            nc.scalar.activation(
                                ht2[:, k], ps1[:, k, 0:TILE], relu,
                                bias=cs[:, g:g + 1], scale=1.0)

            # unit = 4 tiles = 2 pairs.  Software-pipelined with a 1-unit
            # MM2 lag: unit u's MM2s are emitted AFTER unit u+1's MM1s so
            # drains never head-of-line-block the next unit's MM1s on the
            # in-order PE queue.
            def emit_mm1(u):
                ht = hp.tile([H, 4, TILE], mdt, tag="ht", name="ht")
                for pk in range(2):
                    p = 2 * u + pk
                    ps1 = ppp.tile([128, 2, 512], f32, tag="pp", name="ps1")
                    nc.tensor.matmul(ps1[:, 0, 0:TILE], w1s, x_slice(2 * p),
                                     start=True, stop=True)
                    nc.tensor.matmul(ps1[:, 1, 0:TILE], w1s,
                                     x_slice(2 * p + 1),
                                     start=True, stop=True)
                    # drains split strictly ACT/DVE
                    drain(ht[:, 2 * pk:2 * pk + 2], ps1, p,
                          on_dve=(pk == 1))
                return ht

            ps2_tiles = {}

            def emit_mm2(u, ht):
                ps2 = ps2p.tile([128, 512], f32, tag="ps2", name="ps2")
                ps2_tiles[u] = ps2
                for t in range(GRP):
                    nc.tensor.matmul(ps2[32 * t:32 * t + 32, 0:TILE],
                                     w2s, ht[:, t], start=True, stop=True,
                                     tile_position=(0, 32 * t))

            def emit_copy_dma(u):
                """fp16(ps2 of unit u) -> ot [128, 448] -> outP, sync ring."""
                ps2 = ps2_tiles.pop(u)
                ot = op.tile([128, TILE], f16, tag="ot", name="ot")
                # any-engine: the Tile scheduler gap-fills the idler of
                # ACT/DVE, auto-balancing the evacuation load per unit
                nc.any.tensor_copy(ot, ps2[:, 0:TILE])
                nc.sync.dma_start(out=outP[:, u * TILE:(u + 1) * TILE],
                                  in_=ot)

            LAG = int(os.environ.get("KERNEL_MM2_LAG", "1"))
            pend: list = []
            for u in range(NGRP):
                pend.append((u, emit_mm1(u)))
                if len(pend) > LAG:
                    uu, hts = pend.pop(0)
                    emit_mm2(uu, hts)
                    if uu >= 1:
                        emit_copy_dma(uu - 1)
            # tail: copy(8) runs on ACT while MM2(9) runs on the PE
            for uu, hts in pend:
                emit_copy_dma(uu - 1)
                emit_mm2(uu, hts)
            emit_copy_dma(NGRP - 1)

    nc.compile()
    return nc


def _get_nc():
    if "nc" not in _cache:
        _cache["nc"] = _build_bass()
    return _cache["nc"]


# ------------------------------------------------------------ numpy fallback
def _numpy_reference(node_features, global_features, W1, b1, W2, b2,
                     cube_mask, batch, move_mask):
    g_exp = global_features[batch]
    x = np.concatenate([node_features, g_exp], axis=-1)
    h = np.maximum(x @ W1.T + b1, 0.0)
    scores = h @ W2.T + b2
    cm = cube_mask.astype(np.int64)
    excl = np.cumsum(cm) - cm
    seg_base = np.full(B, np.iinfo(np.int64).max)
    np.minimum.at(seg_base, batch, excl)
    rank = excl - seg_base[batch]
    valid = cube_mask & (rank < MAXC)
    logits = np.full((B, MAXC, M), NEG, dtype=np.float32)
    logits[batch[valid], rank[valid]] = scores[valid]
    logits = np.where(move_mask, logits, NEG)
    return logits.reshape(B, -1).astype(np.float32)


# ------------------------------------------------------------------- kernel
def kernel(node_features, global_features, W1, b1, W2, b2,
           cube_mask, batch, move_mask):
    node_features = np.ascontiguousarray(np.asarray(node_features, np.float32))
    global_features = np.asarray(global_features, np.float32)
    W1 = np.asarray(W1, np.float32)
    b1 = np.asarray(b1, np.float32)
    W2 = np.asarray(W2, np.float32)
    b2 = np.asarray(b2, np.float32)
    cube_mask = np.asarray(cube_mask, bool)
    batch = np.asarray(batch, np.int64)
    move_mask = np.asarray(move_mask, bool)

    # --- ranks: cube nodes of each graph fill slots 0..ncubes-1 in order
    cube_idx = np.flatnonzero(cube_mask)
    g = batch[cube_idx]
    counts = np.bincount(g, minlength=B)
    if counts.max() > PAD:
        # a graph overflows the device slot capacity: bail to exact numpy
        return _numpy_reference(node_features, global_features, W1, b1, W2,
                                b2, cube_mask, batch, move_mask)
    starts = np.concatenate(([0], np.cumsum(counts)[:-1]))
    within = np.arange(cube_idx.size, dtype=np.int64) - starts[g]

    if MM_DTYPE == "bf16":
        import ml_dtypes
        mm_np = ml_dtypes.bfloat16
    elif MM_DTYPE == "fp16":
        mm_np = np.float16
    else:
        mm_np = np.float32

    Xslot = np.zeros((B, PAD, D), mm_np)
    Xslot[g, within] = node_features[cube_idx].astype(mm_np)
    slotvalid = np.zeros((B, PAD), bool)
    slotvalid[g, within] = True

    # --- fold the global-feature contribution + b1 into a per-graph bias
    cb = (global_features @ W1[:, D:].T + b1).astype(np.float32)    # [B, H]

    w1T = np.ascontiguousarray(W1[:, :D].T.astype(mm_np))           # [128,128]
    w2T = np.zeros((H, 32), mm_np)
    w2T[:, :M] = W2.T.astype(mm_np)

    nc = _get_nc()
    in_maps = []
    for c in range(NCORES):
        gs = slice(c * BL, (c + 1) * BL)
        xT = np.ascontiguousarray(Xslot[gs].reshape(S, D).T)        # [128, S]
        im = {
            "w1T": w1T,
            "w2T": w2T,
            "cT": np.ascontiguousarray(cb[gs].T),
        }
        for ch in range(NCHUNK):
            im[f"x{ch}"] = np.ascontiguousarray(
                xT[:, CHUNK_OFF[ch]:CHUNK_OFF[ch + 1]])
        in_maps.append(im)

    from concourse.bass_utils import run_bass_kernel_spmd
    res = run_bass_kernel_spmd(nc, in_maps, list(range(NCORES)))
    _cache["last_result"] = res

    # --- unshard: raw fp16 scores -> fp32, mask applied host-side
    mask_eff = move_mask[:, :PAD, :] & slotvalid[:, :, None]        # [B,PAD,M]
    scores = np.empty((B, PAD, M), dtype=np.float32)
    for c in range(NCORES):
        gs = slice(c * BL, (c + 1) * BL)
        outP = np.asarray(res.results[c]["outP"])
        o = outP.reshape(GRP, 32, NGRP, TILE)[:, :M]                # t,m,j,col
        scores[gs] = o.transpose(2, 0, 3, 1).reshape(
            BL, PAD, M).astype(np.float32)

    if np.any(b2):
        scores += b2[None, None, :].astype(np.float32)

    full = np.full((B, MAXC, M), NEG, dtype=np.float32)
    full[:, :PAD] = np.where(mask_eff, scores, NEG)
    return full.reshape(B, MAXC * M)
